# revision 3
# baseline (speedup 1.0000x reference)
"""ANFIS Trainium2 kernel (8 NeuronCores, Bass/Tile).

Math (reference):
  mfs[b,i,j] = exp(-(x[b,i]-centers[i,j])^2 / (2*widths[i,j]^2))   [1024,8,4]
  w[b,r]     = prod_i mfs[b,i,idx_i(r)]    r in [0, 4^8=65536), i0 slowest
  w        <- w / sum_r w
  out[b,n]   = sum_r w[b,r] * ([x[b],1] . rule_params[r,:,n])      [1024,16]

Structure: w = wA (x) wB with wA over dims 0..2 (64 vals, split 8 rA per
core) and wB over dims 3..7 (1024 vals); r = rA*1024 + rB.  Denominator
factorizes: sum_r w = prod_i (sum_j mfs[b,i,j]).

Per core:  psum[b, rA, i*16+n] = sum_rB wB[b,rB] rp[rA*1024+rB, i*16+n]
(bf16 matmuls, rB contracted on partitions, kt = 8 k-tiles), evacuated as
psum * G with G[b, rA*9+i] = wA[b,rA]/denom[b] * xb[b,i], tree-summed over
rA and strided-reduced over i.  Core partials summed on host.

v2 schedule:
  - wB^T via XBAR DMA transposes (dma_start_transpose) for b-tiles 1..7;
    b-tile 0 on the PE (identity matmul) to dodge the XBAR's ~2.5us
    trigger+sem latency on the pipeline head.
  - PE warm-up: dummy matmuls on a zeroed tile (memset on DVE at queue
    head) release the HAM clock gate before the first real matmul.
  - Explicit stage ordering on DVE via add_dep_helper — the tile scheduler
    otherwise interleaves independent chains and starves the critical
    bt0 -> w3456 -> w3s -> transpose path.
  - j-scales (w3s = w3456 * mfs7j) for bt0..2 on DVE (bf16 rate), bt3..7
    on ACT (ACT runs 1.2 GHz: 490ns per op, fine off the critical path).
  - Evacuation xsc/tree in bf16; out DMAs on the sync queue after XBARs.
"""

import sys

sys.path.insert(0, "/opt/trn_rl_repo")

import numpy as np

import concourse.bacc as bacc
import concourse.tile as tile
import concourse.mybir as mybir
from concourse.ap import AP
from concourse.bass_utils import run_bass_kernel_spmd


F32 = mybir.dt.float32
BF16 = mybir.dt.bfloat16
MULT = mybir.AluOpType.mult
ADD = mybir.AluOpType.add
SUB = mybir.AluOpType.subtract
EXP = mybir.ActivationFunctionType.Exp
AXX = mybir.AxisListType.X

N_CORES = 8
B = 1024
BT = 8          # batch tiles of 128
D = 8           # input dims
DX = D + 1      # xb width (x plus ones column)
M = 4           # membership fns per dim
NO = 16         # outputs
C = DX * NO                 # 144
NRA = 64        # 4^3 (dims 0..2)
RA_LOC = NRA // N_CORES     # 8 local rA per core
NRB = 1024      # 4^5 (dims 3..7)
KT = 8          # rB partition tiles of 128
GROUPS = [(0, 3), (3, 3), (6, 2)]
SC = RA_LOC * C  # 1152
DM = D * M       # 32

N_WARM = 16             # dummy warm-up matmuls (256 cols each)
N_DVE_JSCALE_BT = 3     # b-tiles whose j-scales run on DVE (rest on ACT)

# small slab column layout (fp32); part 1 = critical (xab + mf consts)
O_XAB = 0
O_CB = O_XAB + BT * DX            # 72
O_CW2N = O_CB + DM                # 104
NSM1 = O_CW2N + DM                # 136
O_XA3 = NSM1                      # 136
O_CA3 = O_XA3 + BT * RA_LOC * 3   # 328
O_NWA2 = O_CA3 + RA_LOC * 3       # 352
NSM = O_NWA2 + RA_LOC * 3         # 376


def _v(t, off, dims):
    """Custom free-dim view of a [128, F] SBUF tile AP."""
    part = list(t.ap[0])
    return AP(
        tensor=t.tensor,
        offset=t.offset + off,
        ap=[part] + [[s, n] for (s, n) in dims],
    )


def build_nc():
    nc = bacc.Bacc("TRN2", target_bir_lowering=False, debug=False,
                   num_devices=N_CORES)

    small_d = nc.declare_dram_parameter("small", [128, NSM], F32, isOutput=False)
    eye_d = nc.declare_dram_parameter("eye", [128, 128], BF16, isOutput=False)
    rp_d = nc.declare_dram_parameter("rp", [128, KT * SC], BF16, isOutput=False)
    out_d = nc.declare_dram_parameter("out", [B, NO], F32, isOutput=True)

    with tile.TileContext(nc) as tc:
        with (
            tc.tile_pool(name="const", bufs=1) as cpool,
            tc.tile_pool(name="rp", bufs=1) as rppool,
            tc.tile_pool(name="wbt", bufs=1) as wbtpool,
            tc.tile_pool(name="work", bufs=2) as work,
            tc.tile_pool(name="w3s", bufs=3) as w3spool,
            tc.tile_pool(name="psD", bufs=1, space="PSUM") as psDp,
            tc.tile_pool(name="evac", bufs=3) as evpool,
            tc.tile_pool(name="ps0", bufs=2, space="PSUM") as ps0p,
            tc.tile_pool(name="ps1", bufs=2, space="PSUM") as ps1p,
            tc.tile_pool(name="ps2", bufs=2, space="PSUM") as ps2p,
        ):
            # ---- input DMAs ----
            small = cpool.tile([128, NSM], F32, tag="small")
            eye = cpool.tile([128, 128], BF16, tag="eye")
            rp = rppool.tile([128, KT * SC], BF16, tag="rp")
            zs = cpool.tile([128, 512], BF16, tag="zs")

            nc.sync.dma_start(small[:, 0:NSM1], small_d[:, 0:NSM1])
            nc.sync.dma_start(eye[:], eye_d[:])
            nc.sync.dma_start(small[:, NSM1:NSM], small_d[:, NSM1:NSM])
            # rp chunk queues: kt0/kt1 scalar (earliest need), kt2-4 sync,
            # kt5-7 gpsimd (SWDGE)
            for kt, eng in zip(range(KT), (nc.scalar, nc.scalar, nc.sync,
                                           nc.sync, nc.sync, nc.gpsimd,
                                           nc.gpsimd, nc.gpsimd)):
                eng.dma_start(rp[:, kt * SC:(kt + 1) * SC],
                              rp_d[:, kt * SC:(kt + 1) * SC])

            xab = small[:, O_XAB:O_XAB + BT * DX]
            cb = small[:, O_CB:O_CB + DM]
            cw2n = small[:, O_CW2N:O_CW2N + DM]
            xA3 = small[:, O_XA3:O_XA3 + BT * RA_LOC * 3]
            cA3 = small[:, O_CA3:O_CA3 + RA_LOC * 3]
            nwA2 = small[:, O_NWA2:O_NWA2 + RA_LOC * 3]

            # ---- PE warm-up: zero tile (DVE memset, no deps) + dummies ----
            nc.vector.memset(zs[:], 0)
            psD = [psDp.tile([128, 512], F32, tag="psD0", name="psD0"),
                   psDp.tile([128, 512], F32, tag="psD1", name="psD1")]
            for i in range(N_WARM):
                nc.tensor.matmul(psD[i % 2][:, 0:256], zs[:, 0:128],
                                 zs[:, 0:256], start=True, stop=True)

            # DVE stage chain: force scheduler to respect emission order
            last_dve = [None]

            def dve(op_fn, *args, **kwargs):
                i = op_fn(*args, **kwargs)
                if last_dve[0] is not None:
                    tile.add_dep_helper(i.ins, last_dve[0].ins, sync=False,
                                        reason="dve stage order")
                last_dve[0] = i
                return i

            mfs = cpool.tile([128, BT * DM], F32, tag="mfs")
            dif = work.tile([128, BT * DM], F32, tag="dif")
            d2 = work.tile([128, BT * DM], F32, tag="d2")
            d2s = work.tile([128, BT * DM], F32, tag="d2s")

            def mfs_chain(off, nbt, xoff):
                dve(nc.vector.tensor_tensor,
                    _v(dif[:], off, [(DM, nbt), (M, D), (1, M)]),
                    _v(xab, xoff, [(DX, nbt), (1, D), (0, M)]),
                    _v(cb, 0, [(0, nbt), (M, D), (1, M)]),
                    op=SUB)
                dve(nc.vector.tensor_tensor,
                    _v(d2[:], off, [(1, nbt * DM)]),
                    _v(dif[:], off, [(1, nbt * DM)]),
                    _v(dif[:], off, [(1, nbt * DM)]), op=MULT)
                dve(nc.vector.tensor_tensor,
                    _v(d2s[:], off, [(DM, nbt), (1, DM)]),
                    _v(d2[:], off, [(DM, nbt), (1, DM)]),
                    _v(cw2n, 0, [(0, nbt), (1, DM)]), op=MULT)
                nc.scalar.activation(
                    _v(mfs[:], off, [(1, nbt * DM)]),
                    _v(d2s[:], off, [(1, nbt * DM)]), EXP, scale=-1.0)

            w34 = work.tile([128, BT * 16], BF16, tag="w34")
            w56 = work.tile([128, BT * 16], BF16, tag="w56")
            w3456 = cpool.tile([128, BT * 256], BF16, tag="w3456")

            def w_chain(off, nbt):
                dve(nc.vector.tensor_tensor,
                    _v(w34[:], off * 16, [(16, nbt), (M, M), (1, M)]),
                    _v(mfs[:], off * DM + 3 * M, [(DM, nbt), (1, M), (0, M)]),
                    _v(mfs[:], off * DM + 4 * M, [(DM, nbt), (0, M), (1, M)]),
                    op=MULT)
                dve(nc.vector.tensor_tensor,
                    _v(w56[:], off * 16, [(16, nbt), (M, M), (1, M)]),
                    _v(mfs[:], off * DM + 5 * M, [(DM, nbt), (1, M), (0, M)]),
                    _v(mfs[:], off * DM + 6 * M, [(DM, nbt), (0, M), (1, M)]),
                    op=MULT)
                dve(nc.vector.tensor_tensor,
                    _v(w3456[:], off * 256, [(256, nbt), (16, 16), (1, 16)]),
                    _v(w34[:], off * 16, [(16, nbt), (1, 16), (0, 16)]),
                    _v(w56[:], off * 16, [(16, nbt), (0, 16), (1, 16)]),
                    op=MULT)

            wbt = wbtpool.tile([128, KT * B], BF16, tag="wbt")

            def jscales(bt, on_dve):
                w3sall = w3spool.tile([128, 1024], BF16, tag="w3s",
                                      name="w3sall")
                for j in range(M):
                    dst = w3sall[:, j * 256:(j + 1) * 256]
                    src = w3456[:, bt * 256:(bt + 1) * 256]
                    sc = mfs[:, bt * DM + 7 * M + j: bt * DM + 7 * M + j + 1]
                    if on_dve:
                        dve(nc.vector.tensor_scalar_mul, dst, src, sc)
                    else:
                        nc.scalar.mul(dst, src, sc)
                return w3sall

            # ---- stage S1/S2/S3: bt0 chain -> PE transposes ----
            mfs_chain(0, 1, 0)
            w_chain(0, 1)
            w3s0 = jscales(0, on_dve=True)
            psD = [psDp.tile([128, 512], F32, tag="psD0", name="psD0"),
                   psDp.tile([128, 512], F32, tag="psD1", name="psD1")]
            for j in range(M):
                for qh in range(2):
                    kt = 2 * j + qh
                    m, t = kt // 4, kt % 4
                    nc.tensor.matmul(
                        psD[m][:, t * 128:(t + 1) * 128],
                        w3s0[:, kt * 128:(kt + 1) * 128], eye[:],
                        start=True, stop=True)
            for m in range(2):
                nc.scalar.copy(
                    _v(wbt[:], (4 * m) * B + 0 * 128, [(B, 4), (1, 128)]),
                    psD[m][:])

            # ---- S4/S5/S6: bt1 chain -> XBAR ----
            mfs_chain(DM, 1, DX)
            w_chain(1, 1)
            w3s1 = jscales(1, on_dve=True)
            nc.sync.dma_start_transpose(
                _v(wbt[:], 1 * 128, [(B, KT), (1, 128)]), w3s1[:])

            # ---- main matmuls emitted per bt; builds interleave ----
            def main_mms(bt, ps):
                for kt in range(KT):
                    lhsT = wbt[:, kt * B + bt * 128: kt * B + (bt + 1) * 128]
                    for g, (r0, nr) in enumerate(GROUPS):
                        nc.tensor.matmul(
                            ps[g][:], lhsT,
                            _v(rp[:], (kt * RA_LOC + r0) * C,
                               [(C, nr), (1, C)]),
                            start=(kt == 0), stop=(kt == KT - 1))

            def alloc_ps():
                return [
                    ps0p.tile([128, GROUPS[0][1] * C], F32, tag="ps0", name="ps0"),
                    ps1p.tile([128, GROUPS[1][1] * C], F32, tag="ps1", name="ps1"),
                    ps2p.tile([128, GROUPS[2][1] * C], F32, tag="ps2", name="ps2")]

            ps_bt = [None] * BT
            ps_bt[0] = alloc_ps()
            main_mms(0, ps_bt[0])

            # ---- S7: wA chain ----
            NA = BT * RA_LOC * 3  # 192
            dA = work.tile([128, NA], F32, tag="dA")
            dve(nc.vector.tensor_tensor,
                dA[:], xA3, _v(cA3, 0, [(0, BT), (1, RA_LOC * 3)]), op=SUB)
            d2A = work.tile([128, NA], F32, tag="d2A")
            dve(nc.vector.tensor_tensor, d2A[:], dA[:], dA[:], op=MULT)
            d2sA = work.tile([128, NA], F32, tag="d2sA")
            dve(nc.vector.tensor_tensor,
                d2sA[:], d2A[:], _v(nwA2, 0, [(0, BT), (1, RA_LOC * 3)]),
                op=MULT)
            eA = work.tile([128, BT * RA_LOC], F32, tag="eA")
            dve(nc.vector.reduce_sum,
                eA[:], _v(d2sA[:], 0, [(3, BT * RA_LOC), (1, 3)]), axis=AXX)
            wA = cpool.tile([128, BT * RA_LOC], F32, tag="wA")
            nc.scalar.activation(wA[:], eA[:], EXP, scale=-1.0)

            # ---- S8: bulk mfs (bt2..7) ----
            mfs_chain(2 * DM, BT - 2, 2 * DX)

            # ---- S9: denominator + wAn + G ----
            s = work.tile([128, BT * D], F32, tag="s")
            dve(nc.vector.reduce_sum,
                s[:], _v(mfs[:], 0, [(M, BT * D), (1, M)]), axis=AXX)
            p1 = work.tile([128, BT * 4], F32, tag="p1")
            dve(nc.vector.tensor_tensor,
                p1[:], _v(s[:], 0, [(D, BT), (1, 4)]),
                _v(s[:], 4, [(D, BT), (1, 4)]), op=MULT)
            p2 = work.tile([128, BT * 2], F32, tag="p2")
            dve(nc.vector.tensor_tensor,
                p2[:], _v(p1[:], 0, [(4, BT), (1, 2)]),
                _v(p1[:], 2, [(4, BT), (1, 2)]), op=MULT)
            p3 = work.tile([128, BT], F32, tag="p3")
            dve(nc.vector.tensor_tensor,
                p3[:], _v(p2[:], 0, [(2, BT)]), _v(p2[:], 1, [(2, BT)]),
                op=MULT)
            invd = cpool.tile([128, BT], F32, tag="invd")
            dve(nc.vector.reciprocal, invd[:], p3[:])
            wAn = cpool.tile([128, BT * RA_LOC], F32, tag="wAn")
            dve(nc.vector.tensor_tensor,
                wAn[:],
                _v(wA[:], 0, [(RA_LOC, BT), (1, RA_LOC)]),
                _v(invd[:], 0, [(1, BT), (0, RA_LOC)]), op=MULT)
            Gall = cpool.tile([128, BT * RA_LOC * DX], F32, tag="Gall")
            dve(nc.vector.tensor_tensor,
                Gall[:],
                _v(wAn[:], 0, [(RA_LOC, BT), (1, RA_LOC), (0, DX)]),
                _v(xab, 0, [(DX, BT), (0, RA_LOC), (1, DX)]), op=MULT)

            # ---- S10: w chains bt2..7 + j-scales + XBARs ----
            w_chain(2, BT - 2)
            for bt in range(2, BT):
                w3sall = jscales(bt, on_dve=(bt < N_DVE_JSCALE_BT))
                nc.sync.dma_start_transpose(
                    _v(wbt[:], bt * 128, [(B, KT), (1, 128)]), w3sall[:])

            # ---- S11: remaining mains + evacs ----
            def evac(bt, ps):
                xsc = evpool.tile([128, SC], BF16, tag="xsc")
                for g, (r0, nr) in enumerate(GROUPS):
                    dve(nc.vector.tensor_tensor,
                        xsc[:, r0 * C:(r0 + nr) * C], ps[g][:],
                        _v(Gall[:], bt * RA_LOC * DX + r0 * DX,
                           [(DX, nr), (1, DX), (0, NO)]),
                        op=MULT)
                th = evpool.tile([128, 4 * C], BF16, tag="th")
                dve(nc.vector.tensor_tensor,
                    th[:], xsc[:, 0:4 * C], xsc[:, 4 * C:8 * C], op=ADD)
                th2 = evpool.tile([128, 2 * C], BF16, tag="th2")
                dve(nc.vector.tensor_tensor,
                    th2[:], th[:, 0:2 * C], th[:, 2 * C:4 * C], op=ADD)
                th3 = evpool.tile([128, C], BF16, tag="th3")
                dve(nc.vector.tensor_tensor,
                    th3[:], th2[:, 0:C], th2[:, C:2 * C], op=ADD)
                ob = evpool.tile([128, NO], F32, tag="ob")
                dve(nc.vector.reduce_sum,
                    ob[:], _v(th3[:], 0, [(1, NO), (NO, DX)]), axis=AXX)
                return ob

            obs = [None] * BT
            for bt in range(1, BT):
                ps_bt[bt] = alloc_ps()
                main_mms(bt, ps_bt[bt])
                obs[bt - 1] = evac(bt - 1, ps_bt[bt - 1])
            obs[BT - 1] = evac(BT - 1, ps_bt[BT - 1])

            for bt in range(BT):
                nc.sync.dma_start(out_d[bt * 128:(bt + 1) * 128, :], obs[bt][:])

    nc.compile()
    return nc


_NC_CACHE = None


def _get_nc():
    global _NC_CACHE
    if _NC_CACHE is None:
        _NC_CACHE = build_nc()
    return _NC_CACHE


def _prep_in_maps(x, centers, widths, rule_params):
    import ml_dtypes

    x = np.asarray(x, np.float32)
    centers = np.asarray(centers, np.float32)
    widths = np.asarray(widths, np.float32)
    rule_params = np.asarray(rule_params, np.float32)

    # xab[p, bt*9+i] = x[bt*128+p, i] for i<8; 1.0 at i=8
    xab = np.ones((128, BT, DX), np.float32)
    xab[:, :, :D] = x.reshape(BT, 128, D).transpose(1, 0, 2)
    xab = xab.reshape(128, BT * DX)
    cb = np.broadcast_to(centers.reshape(1, DM), (128, DM))
    cw2n = np.broadcast_to((1.0 / (2.0 * widths * widths)).reshape(1, DM),
                           (128, DM))
    eye = np.eye(128, dtype=ml_dtypes.bfloat16)

    # xA3[p, bt*24 + r*3 + k] = x[bt*128+p, k]
    xA3 = np.broadcast_to(
        x.reshape(BT, 128, D).transpose(1, 0, 2)[:, :, None, 0:3],
        (128, BT, RA_LOC, 3)).reshape(128, BT * RA_LOC * 3)

    # rule_params rows r = rA*1024 + q*4 + j -> per core [p, kt, rA, c]
    # with row order rB' = j*256 + q, kt = rB' tile of 128.
    rp4 = rule_params.reshape(NRA, 256, M, C).transpose(0, 2, 1, 3)
    rp4 = rp4.reshape(NRA, NRB, C)

    in_maps = []
    for c in range(N_CORES):
        ra0 = c * RA_LOC
        idx = np.empty((RA_LOC, 3), np.int64)
        for r in range(RA_LOC):
            ra = ra0 + r
            idx[r] = [(ra >> 4) & 3, (ra >> 2) & 3, ra & 3]
        k = np.arange(3)
        cA = centers[k[None, :], idx]
        wtA = widths[k[None, :], idx]
        cA3 = np.broadcast_to(cA.reshape(1, RA_LOC * 3), (128, RA_LOC * 3))
        nwA2 = np.broadcast_to(
            (1.0 / (2.0 * wtA * wtA)).reshape(1, RA_LOC * 3),
            (128, RA_LOC * 3))
        small = np.ascontiguousarray(
            np.concatenate([xab, cb, cw2n, xA3, cA3, nwA2], axis=1,
                           dtype=np.float32))

        rp_c = rp4[ra0:ra0 + RA_LOC]                     # [8, 1024, 144]
        rp_c = rp_c.reshape(RA_LOC, KT, 128, C).transpose(2, 1, 0, 3)
        rp_c = np.ascontiguousarray(
            rp_c.reshape(128, KT * SC)).astype(ml_dtypes.bfloat16)

        in_maps.append({"small": small, "eye": eye, "rp": rp_c})
    return in_maps


def kernel(x, centers, widths, rule_params, _trace=False):
    nc = _get_nc()
    in_maps = _prep_in_maps(x, centers, widths, rule_params)
    res = run_bass_kernel_spmd(nc, in_maps, core_ids=list(range(N_CORES)),
                               trace=_trace)
    out = np.sum([np.asarray(res.results[c]["out"], np.float32)
                  for c in range(N_CORES)], axis=0)
    if _trace:
        kernel._last_exec_time_ns = res.exec_time_ns
        kernel._last_results = res
    return out


# revision 7
# speedup vs baseline: 1.0827x; 1.0827x over previous
"""ANFIS Trainium2 kernel (8 NeuronCores, Bass/Tile).

Math (reference):
  mfs[b,i,j] = exp(-(x[b,i]-centers[i,j])^2 / (2*widths[i,j]^2))   [1024,8,4]
  w[b,r]     = prod_i mfs[b,i,idx_i(r)]    r in [0, 4^8=65536), i0 slowest
  w        <- w / sum_r w
  out[b,n]   = sum_r w[b,r] * ([x[b],1] . rule_params[r,:,n])      [1024,16]

Structure: w = wA (x) wB with wA over dims 0..2 (64 vals, split 8 rA per
core) and wB over dims 3..7 (1024 vals); r = rA*1024 + rB.  Denominator
factorizes: sum_r w = prod_i (sum_j mfs[b,i,j]).

Per core:  psum[b, rA, i*16+n] = sum_rB wB[b,rB] rp[rA*1024+rB, i*16+n]
(bf16 matmuls, rB contracted on partitions, kt = 8 k-tiles), evacuated as
psum * G with G[b, rA*9+i] = wA[b,rA]/denom[b] * xb[b,i], tree-summed over
rA and strided-reduced over i.  Core partials summed on host.

v2 schedule:
  - wB^T via XBAR DMA transposes (dma_start_transpose) for b-tiles 1..7;
    b-tile 0 on the PE (identity matmul) to dodge the XBAR's ~2.5us
    trigger+sem latency on the pipeline head.
  - PE warm-up: dummy matmuls on a zeroed tile (memset on DVE at queue
    head) release the HAM clock gate before the first real matmul.
  - Explicit stage ordering on DVE via add_dep_helper — the tile scheduler
    otherwise interleaves independent chains and starves the critical
    bt0 -> w3456 -> w3s -> transpose path.
  - j-scales (w3s = w3456 * mfs7j) for bt0..2 on DVE (bf16 rate), bt3..7
    on ACT (ACT runs 1.2 GHz: 490ns per op, fine off the critical path).
  - Evacuation xsc/tree in bf16; out DMAs on the sync queue after XBARs.
"""

import sys

sys.path.insert(0, "/opt/trn_rl_repo")

import numpy as np

import concourse.bacc as bacc
import concourse.tile as tile
import concourse.mybir as mybir
from concourse.ap import AP
from concourse.bass_utils import run_bass_kernel_spmd


F32 = mybir.dt.float32
BF16 = mybir.dt.bfloat16
MULT = mybir.AluOpType.mult
ADD = mybir.AluOpType.add
SUB = mybir.AluOpType.subtract
EXP = mybir.ActivationFunctionType.Exp
AXX = mybir.AxisListType.X

N_CORES = 8
B = 1024
BT = 8          # batch tiles of 128
D = 8           # input dims
DX = D + 1      # xb width (x plus ones column)
M = 4           # membership fns per dim
NO = 16         # outputs
C = DX * NO                 # 144
NRA = 64        # 4^3 (dims 0..2)
RA_LOC = NRA // N_CORES     # 8 local rA per core
NRB = 1024      # 4^5 (dims 3..7)
KT = 8          # rB partition tiles of 128
GROUPS = [(0, 3), (3, 3), (6, 2)]
SC = RA_LOC * C  # 1152
DM = D * M       # 32

N_WARM = 8              # dummy warm-up matmuls (256 cols each)
N_DVE_JSCALE_BT = 3     # b-tiles whose j-scales run on DVE (rest on ACT)

# small slab column layout (fp32); part 1 = critical (xab + mf consts)
O_XAB = 0
O_CB = O_XAB + BT * DX            # 72
O_CW2N = O_CB + DM                # 104
NSM1 = O_CW2N + DM                # 136
O_XA3 = NSM1                      # 136
O_CA3 = O_XA3 + BT * RA_LOC * 3   # 328
O_NWA2 = O_CA3 + RA_LOC * 3       # 352
NSM = O_NWA2 + RA_LOC * 3         # 376


def _v(t, off, dims):
    """Custom free-dim view of a [128, F] SBUF tile AP."""
    part = list(t.ap[0])
    return AP(
        tensor=t.tensor,
        offset=t.offset + off,
        ap=[part] + [[s, n] for (s, n) in dims],
    )


def build_nc():
    nc = bacc.Bacc("TRN2", target_bir_lowering=False, debug=False,
                   num_devices=N_CORES)

    small_d = nc.declare_dram_parameter("small", [128, NSM], F32, isOutput=False)
    eye_d = nc.declare_dram_parameter("eye", [128, 128], BF16, isOutput=False)
    rp_d = nc.declare_dram_parameter("rp", [128, KT * SC], BF16, isOutput=False)
    out_d = nc.declare_dram_parameter("out", [B, NO], F32, isOutput=True)

    with tile.TileContext(nc) as tc:
        with (
            tc.tile_pool(name="const", bufs=1) as cpool,
            tc.tile_pool(name="rp", bufs=1) as rppool,
            tc.tile_pool(name="wbt", bufs=1) as wbtpool,
            tc.tile_pool(name="work", bufs=2) as work,
            tc.tile_pool(name="w3s", bufs=3) as w3spool,
            tc.tile_pool(name="psD", bufs=1, space="PSUM") as psDp,
            tc.tile_pool(name="evac", bufs=3) as evpool,
            tc.tile_pool(name="ps0", bufs=2, space="PSUM") as ps0p,
            tc.tile_pool(name="ps1", bufs=2, space="PSUM") as ps1p,
            tc.tile_pool(name="ps2", bufs=2, space="PSUM") as ps2p,
        ):
            # ---- input DMAs ----
            small = cpool.tile([128, NSM], F32, tag="small")
            eye = cpool.tile([128, 128], BF16, tag="eye")
            rp = rppool.tile([128, KT * SC], BF16, tag="rp")
            zs = cpool.tile([128, 512], BF16, tag="zs")

            nc.sync.dma_start(small[:, 0:NSM1], small_d[:, 0:NSM1])
            nc.sync.dma_start(eye[:], eye_d[:])
            nc.sync.dma_start(small[:, NSM1:NSM], small_d[:, NSM1:NSM])
            # rp chunk queues: kt0/kt1 scalar (earliest need), kt2-4 sync,
            # kt5-7 gpsimd (SWDGE)
            for kt, eng in zip(range(KT), (nc.scalar, nc.scalar, nc.sync,
                                           nc.sync, nc.sync, nc.gpsimd,
                                           nc.gpsimd, nc.gpsimd)):
                eng.dma_start(rp[:, kt * SC:(kt + 1) * SC],
                              rp_d[:, kt * SC:(kt + 1) * SC])

            xab = small[:, O_XAB:O_XAB + BT * DX]
            cb = small[:, O_CB:O_CB + DM]
            cw2n = small[:, O_CW2N:O_CW2N + DM]
            xA3 = small[:, O_XA3:O_XA3 + BT * RA_LOC * 3]
            cA3 = small[:, O_CA3:O_CA3 + RA_LOC * 3]
            nwA2 = small[:, O_NWA2:O_NWA2 + RA_LOC * 3]

            # ---- PE warm-up: zero tile (DVE memset, no deps) + dummies ----
            nc.vector.memset(zs[:], 0)
            psD = [psDp.tile([128, 512], F32, tag="psD0", name="psD0"),
                   psDp.tile([128, 512], F32, tag="psD1", name="psD1")]
            for i in range(N_WARM):
                nc.tensor.matmul(psD[i % 2][:, 0:256], zs[:, 0:128],
                                 zs[:, 0:256], start=True, stop=True)

            # DVE stage chain: force scheduler to respect emission order
            last_dve = [None]

            def dve(op_fn, *args, **kwargs):
                i = op_fn(*args, **kwargs)
                if last_dve[0] is not None:
                    tile.add_dep_helper(i.ins, last_dve[0].ins, sync=False,
                                        reason="dve stage order")
                last_dve[0] = i
                return i

            mfs = cpool.tile([128, BT * DM], F32, tag="mfs")
            dif = work.tile([128, BT * DM], F32, tag="dif")
            d2 = work.tile([128, BT * DM], F32, tag="d2")
            d2s = work.tile([128, BT * DM], F32, tag="d2s")

            def mfs_chain(off, nbt, xoff):
                dve(nc.vector.tensor_tensor,
                    _v(dif[:], off, [(DM, nbt), (M, D), (1, M)]),
                    _v(xab, xoff, [(DX, nbt), (1, D), (0, M)]),
                    _v(cb, 0, [(0, nbt), (M, D), (1, M)]),
                    op=SUB)
                dve(nc.vector.tensor_tensor,
                    _v(d2[:], off, [(1, nbt * DM)]),
                    _v(dif[:], off, [(1, nbt * DM)]),
                    _v(dif[:], off, [(1, nbt * DM)]), op=MULT)
                dve(nc.vector.tensor_tensor,
                    _v(d2s[:], off, [(DM, nbt), (1, DM)]),
                    _v(d2[:], off, [(DM, nbt), (1, DM)]),
                    _v(cw2n, 0, [(0, nbt), (1, DM)]), op=MULT)
                nc.scalar.activation(
                    _v(mfs[:], off, [(1, nbt * DM)]),
                    _v(d2s[:], off, [(1, nbt * DM)]), EXP, scale=-1.0)

            w34 = work.tile([128, BT * 16], BF16, tag="w34")
            w56 = work.tile([128, BT * 16], BF16, tag="w56")
            w3456 = cpool.tile([128, BT * 256], BF16, tag="w3456")

            def w_chain(off, nbt):
                dve(nc.vector.tensor_tensor,
                    _v(w34[:], off * 16, [(16, nbt), (M, M), (1, M)]),
                    _v(mfs[:], off * DM + 3 * M, [(DM, nbt), (1, M), (0, M)]),
                    _v(mfs[:], off * DM + 4 * M, [(DM, nbt), (0, M), (1, M)]),
                    op=MULT)
                dve(nc.vector.tensor_tensor,
                    _v(w56[:], off * 16, [(16, nbt), (M, M), (1, M)]),
                    _v(mfs[:], off * DM + 5 * M, [(DM, nbt), (1, M), (0, M)]),
                    _v(mfs[:], off * DM + 6 * M, [(DM, nbt), (0, M), (1, M)]),
                    op=MULT)
                dve(nc.vector.tensor_tensor,
                    _v(w3456[:], off * 256, [(256, nbt), (16, 16), (1, 16)]),
                    _v(w34[:], off * 16, [(16, nbt), (1, 16), (0, 16)]),
                    _v(w56[:], off * 16, [(16, nbt), (0, 16), (1, 16)]),
                    op=MULT)

            wbt = wbtpool.tile([128, KT * B], BF16, tag="wbt")

            def jscales(bt, on_dve):
                w3sall = w3spool.tile([128, 1024], BF16, tag="w3s",
                                      name="w3sall")
                for j in range(M):
                    dst = w3sall[:, j * 256:(j + 1) * 256]
                    src = w3456[:, bt * 256:(bt + 1) * 256]
                    sc = mfs[:, bt * DM + 7 * M + j: bt * DM + 7 * M + j + 1]
                    if on_dve:
                        dve(nc.vector.tensor_scalar_mul, dst, src, sc)
                    else:
                        nc.scalar.mul(dst, src, sc)
                return w3sall

            # ---- S1: mfs for ALL b-tiles, one chain + one exp ----
            mfs_chain(0, BT, 0)

            # ---- S2: per-bt w-chains + j-scales for bt0..2 ----
            # bt0 -> PE transposes (XBAR latency would sit on the head);
            # bt1, bt2 -> XBAR.
            w_chain(0, 1)
            w3s0 = jscales(0, on_dve=True)
            psD = [psDp.tile([128, 512], F32, tag="psD0", name="psD0"),
                   psDp.tile([128, 512], F32, tag="psD1", name="psD1")]
            for j in range(M):
                for qh in range(2):
                    kt = 2 * j + qh
                    m, t = kt // 4, kt % 4
                    nc.tensor.matmul(
                        psD[m][:, t * 128:(t + 1) * 128],
                        w3s0[:, kt * 128:(kt + 1) * 128], eye[:],
                        start=True, stop=True)
            for m in range(2):
                nc.scalar.copy(
                    _v(wbt[:], (4 * m) * B + 0 * 128, [(B, 4), (1, 128)]),
                    psD[m][:])

            for bt in (1, 2):
                w_chain(bt, 1)
                w3sb = jscales(bt, on_dve=True)
                nc.sync.dma_start_transpose(
                    _v(wbt[:], bt * 128, [(B, KT), (1, 128)]), w3sb[:])

            # ---- main matmuls emitted per bt; builds interleave ----
            def main_mms(bt, ps):
                for kt in range(KT):
                    lhsT = wbt[:, kt * B + bt * 128: kt * B + (bt + 1) * 128]
                    for g, (r0, nr) in enumerate(GROUPS):
                        nc.tensor.matmul(
                            ps[g][:], lhsT,
                            _v(rp[:], (kt * RA_LOC + r0) * C,
                               [(C, nr), (1, C)]),
                            start=(kt == 0), stop=(kt == KT - 1))

            def alloc_ps():
                return [
                    ps0p.tile([128, GROUPS[0][1] * C], F32, tag="ps0", name="ps0"),
                    ps1p.tile([128, GROUPS[1][1] * C], F32, tag="ps1", name="ps1"),
                    ps2p.tile([128, GROUPS[2][1] * C], F32, tag="ps2", name="ps2")]

            ps_bt = [None] * BT
            ps_bt[0] = alloc_ps()
            main_mms(0, ps_bt[0])

            # ---- S3: bulk w-chain bt3..7 (j-scales on ACT below) ----
            w_chain(3, BT - 3)

            # ---- S4: wA chain ----
            NA = BT * RA_LOC * 3  # 192
            dA = work.tile([128, NA], F32, tag="dA")
            dve(nc.vector.tensor_tensor,
                dA[:], xA3, _v(cA3, 0, [(0, BT), (1, RA_LOC * 3)]), op=SUB)
            d2A = work.tile([128, NA], F32, tag="d2A")
            dve(nc.vector.tensor_tensor, d2A[:], dA[:], dA[:], op=MULT)
            d2sA = work.tile([128, NA], F32, tag="d2sA")
            dve(nc.vector.tensor_tensor,
                d2sA[:], d2A[:], _v(nwA2, 0, [(0, BT), (1, RA_LOC * 3)]),
                op=MULT)
            eA = work.tile([128, BT * RA_LOC], F32, tag="eA")
            dve(nc.vector.reduce_sum,
                eA[:], _v(d2sA[:], 0, [(3, BT * RA_LOC), (1, 3)]), axis=AXX)
            wA = cpool.tile([128, BT * RA_LOC], F32, tag="wA")
            nc.scalar.activation(wA[:], eA[:], EXP, scale=-1.0)

            # ---- S5: j-scales bt3..7 on ACT + XBARs ----
            for bt in range(3, BT):
                w3sb = jscales(bt, on_dve=False)
                nc.sync.dma_start_transpose(
                    _v(wbt[:], bt * 128, [(B, KT), (1, 128)]), w3sb[:])

            # ---- S6: denominator + wAn + G ----
            s = work.tile([128, BT * D], F32, tag="s")
            dve(nc.vector.reduce_sum,
                s[:], _v(mfs[:], 0, [(M, BT * D), (1, M)]), axis=AXX)
            p1 = work.tile([128, BT * 4], F32, tag="p1")
            dve(nc.vector.tensor_tensor,
                p1[:], _v(s[:], 0, [(D, BT), (1, 4)]),
                _v(s[:], 4, [(D, BT), (1, 4)]), op=MULT)
            p2 = work.tile([128, BT * 2], F32, tag="p2")
            dve(nc.vector.tensor_tensor,
                p2[:], _v(p1[:], 0, [(4, BT), (1, 2)]),
                _v(p1[:], 2, [(4, BT), (1, 2)]), op=MULT)
            p3 = work.tile([128, BT], F32, tag="p3")
            dve(nc.vector.tensor_tensor,
                p3[:], _v(p2[:], 0, [(2, BT)]), _v(p2[:], 1, [(2, BT)]),
                op=MULT)
            invd = cpool.tile([128, BT], F32, tag="invd")
            dve(nc.vector.reciprocal, invd[:], p3[:])
            wAn = cpool.tile([128, BT * RA_LOC], F32, tag="wAn")
            dve(nc.vector.tensor_tensor,
                wAn[:],
                _v(wA[:], 0, [(RA_LOC, BT), (1, RA_LOC)]),
                _v(invd[:], 0, [(1, BT), (0, RA_LOC)]), op=MULT)
            Gall = cpool.tile([128, BT * RA_LOC * DX], BF16, tag="Gall")
            dve(nc.vector.tensor_tensor,
                Gall[:],
                _v(wAn[:], 0, [(RA_LOC, BT), (1, RA_LOC), (0, DX)]),
                _v(xab, 0, [(DX, BT), (0, RA_LOC), (1, DX)]), op=MULT)

            # ---- S7: remaining mains + evacs ----
            def evac(bt, ps):
                xsc = evpool.tile([128, SC], BF16, tag="xsc")
                for g, (r0, nr) in enumerate(GROUPS):
                    dve(nc.vector.tensor_tensor,
                        xsc[:, r0 * C:(r0 + nr) * C], ps[g][:],
                        _v(Gall[:], bt * RA_LOC * DX + r0 * DX,
                           [(DX, nr), (1, DX), (0, NO)]),
                        op=MULT)
                th = evpool.tile([128, 4 * C], BF16, tag="th")
                dve(nc.vector.tensor_tensor,
                    th[:], xsc[:, 0:4 * C], xsc[:, 4 * C:8 * C], op=ADD)
                th2 = evpool.tile([128, 2 * C], BF16, tag="th2")
                dve(nc.vector.tensor_tensor,
                    th2[:], th[:, 0:2 * C], th[:, 2 * C:4 * C], op=ADD)
                th3 = evpool.tile([128, C], BF16, tag="th3")
                dve(nc.vector.tensor_tensor,
                    th3[:], th2[:, 0:C], th2[:, C:2 * C], op=ADD)
                ob = evpool.tile([128, NO], F32, tag="ob")
                dve(nc.vector.reduce_sum,
                    ob[:], _v(th3[:], 0, [(1, NO), (NO, DX)]), axis=AXX)
                return ob

            obs = [None] * BT
            for bt in range(1, BT):
                ps_bt[bt] = alloc_ps()
                main_mms(bt, ps_bt[bt])
                obs[bt - 1] = evac(bt - 1, ps_bt[bt - 1])
            obs[BT - 1] = evac(BT - 1, ps_bt[BT - 1])

            for bt in range(BT):
                nc.sync.dma_start(out_d[bt * 128:(bt + 1) * 128, :], obs[bt][:])

    nc.compile()
    return nc


_NC_CACHE = None


def _get_nc():
    global _NC_CACHE
    if _NC_CACHE is None:
        _NC_CACHE = build_nc()
    return _NC_CACHE


def _prep_in_maps(x, centers, widths, rule_params):
    import ml_dtypes

    x = np.asarray(x, np.float32)
    centers = np.asarray(centers, np.float32)
    widths = np.asarray(widths, np.float32)
    rule_params = np.asarray(rule_params, np.float32)

    # xab[p, bt*9+i] = x[bt*128+p, i] for i<8; 1.0 at i=8
    xab = np.ones((128, BT, DX), np.float32)
    xab[:, :, :D] = x.reshape(BT, 128, D).transpose(1, 0, 2)
    xab = xab.reshape(128, BT * DX)
    cb = np.broadcast_to(centers.reshape(1, DM), (128, DM))
    cw2n = np.broadcast_to((1.0 / (2.0 * widths * widths)).reshape(1, DM),
                           (128, DM))
    eye = np.eye(128, dtype=ml_dtypes.bfloat16)

    # xA3[p, bt*24 + r*3 + k] = x[bt*128+p, k]
    xA3 = np.broadcast_to(
        x.reshape(BT, 128, D).transpose(1, 0, 2)[:, :, None, 0:3],
        (128, BT, RA_LOC, 3)).reshape(128, BT * RA_LOC * 3)

    # rule_params rows r = rA*1024 + q*4 + j -> per core [p, kt, rA, c]
    # with row order rB' = j*256 + q, kt = rB' tile of 128.
    rp4 = rule_params.reshape(NRA, 256, M, C).transpose(0, 2, 1, 3)
    rp4 = rp4.reshape(NRA, NRB, C)

    in_maps = []
    for c in range(N_CORES):
        ra0 = c * RA_LOC
        idx = np.empty((RA_LOC, 3), np.int64)
        for r in range(RA_LOC):
            ra = ra0 + r
            idx[r] = [(ra >> 4) & 3, (ra >> 2) & 3, ra & 3]
        k = np.arange(3)
        cA = centers[k[None, :], idx]
        wtA = widths[k[None, :], idx]
        cA3 = np.broadcast_to(cA.reshape(1, RA_LOC * 3), (128, RA_LOC * 3))
        nwA2 = np.broadcast_to(
            (1.0 / (2.0 * wtA * wtA)).reshape(1, RA_LOC * 3),
            (128, RA_LOC * 3))
        small = np.ascontiguousarray(
            np.concatenate([xab, cb, cw2n, xA3, cA3, nwA2], axis=1,
                           dtype=np.float32))

        rp_c = rp4[ra0:ra0 + RA_LOC]                     # [8, 1024, 144]
        rp_c = rp_c.reshape(RA_LOC, KT, 128, C).transpose(2, 1, 0, 3)
        rp_c = np.ascontiguousarray(
            rp_c.reshape(128, KT * SC)).astype(ml_dtypes.bfloat16)

        in_maps.append({"small": small, "eye": eye, "rp": rp_c})
    return in_maps


def kernel(x, centers, widths, rule_params, _trace=False):
    nc = _get_nc()
    in_maps = _prep_in_maps(x, centers, widths, rule_params)
    res = run_bass_kernel_spmd(nc, in_maps, core_ids=list(range(N_CORES)),
                               trace=_trace)
    out = np.sum([np.asarray(res.results[c]["out"], np.float32)
                  for c in range(N_CORES)], axis=0)
    if _trace:
        kernel._last_exec_time_ns = res.exec_time_ns
        kernel._last_results = res
    return out
